# revision 3
# baseline (speedup 1.0000x reference)
"""AFD loss kernel for 8 TRN2 NeuronCores (Bass/Tile).

Algorithm (matches the reference loss_fn):
  f  = l2norm(features); fa = l2norm(features_adv)
  per-class sums/counts of f via one-hot matmul   (batch-sharded)
  centers_new = where(counts>0, 0.9*centers + 0.1*sums/max(counts,1), centers)
  intra = mean ||f - centers_new[labels]|| + mean ||fa - centers_new[labels]||
        with ||x - c||^2 = 1 - 2 x.c + ||c||^2       (x unit-norm)
  inter = sum_{i<j} relu(1 - ||ci - cj||) / n_pairs   (symmetric full-sum trick)
  loss  = intra - 0.5 * inter

Structure (v13):
  - batch-sharded inputs (features and features_adv passed bf16); centers
    row-sharded on host
  - segment sums via bf16 one-hot matmuls into fp32 PSUM; the count column
    is fused in as column 1024 (f tiles carry a ones column); one-hot spans
    1024 padded classes so pad rows are exact zeros
  - reduction across cores: one fp8 AllToAll of the x16-scaled sums matrix
    (mesh algorithm, ~2x faster than the RDH ReduceScatter it replaces)
    followed by 7 local fp32 vector adds; scale cancels in mean=sums/cnt
  - features_adv loads + norms are emitted in the AllToAll window
  - one merged fp8 AllGather: updated center rows (exact fp32 csq carried as
    bitcast columns) + locally PE-transposed CnT blocks + 0.25x-scaled csq
    row (recovered by a 4.0-valued ones-row matmul)
  - intra: one gathered row per batch tile; dots for BOTH branches via
    fused scalar_tensor_tensor (DVE) and csq of the gathered row via
    Square-accumulate (ACT): d^2 = 1 - 2 dot + ||G||^2 exactly
  - inter: per-rank column blocks of -2*Cn_my @ Cn.T from one contiguous
    stage load; zero pad rows/cols provably contribute 0; symmetric
    full-sum with the C diagonal terms removed in the final formula
  - per-core [intra_sum, inter_sum] partials; host sums 8x2 floats and
    applies the affine formula (the unshard step)
"""

import os
from contextlib import ExitStack

import numpy as np

NCORES = 8
B = 8192
D = 1024
C = 1000
BLOC = B // NCORES          # 1024 rows per core
NB = BLOC // 128            # 8 batch tiles per core
MOM = 0.9
N_PAIRS = C * (C - 1) / 2.0
NCCH = 8                    # class chunks (1024 padded classes)
CPAD = 1024                 # classes padded to full chunks
DBF = D + 8                 # fp8 center row: D data + csq(f32 as 4 fp8) + pad
RW = D + 1                  # reduce row width: sums + count column
AGR = 257                   # AG rows/rank: 128 cn + 128 cnT-stage + 1 csq
SSCALE = 16.0               # fp8 scale for the AllToAll sums payload

_state = {}


def _build():
    import concourse.bacc as bacc
    import concourse.bass as bass
    import concourse.mybir as mybir
    import concourse.tile as tile
    from concourse.masks import make_identity

    fp32 = mybir.dt.float32
    bf16 = mybir.dt.bfloat16
    fp8 = mybir.dt.float8e4
    i32 = mybir.dt.int32
    AF = mybir.ActivationFunctionType
    ALU = mybir.AluOpType
    AX = mybir.AxisListType

    nc = bacc.Bacc("TRN2", target_bir_lowering=False, debug=False,
                   num_devices=NCORES)

    feat = nc.dram_tensor("features", [BLOC, D], bf16, kind="ExternalInput")
    feat_adv = nc.dram_tensor("features_adv", [BLOC, D], bf16,
                              kind="ExternalInput")
    centers_sh = nc.dram_tensor("centers_sh", [128, D], fp32,
                                kind="ExternalInput")
    labels = nc.dram_tensor("labels", [BLOC, 1], i32, kind="ExternalInput")
    labels_g = nc.dram_tensor("labels_g", [128, NB], i32,
                              kind="ExternalInput")
    out = nc.dram_tensor("out", [1, 2], fp32, kind="ExternalOutput")

    with tile.TileContext(nc) as tc:
        with (
            tc.tile_pool(name="const", bufs=1) as constp,
            tc.tile_pool(name="resid", bufs=1) as resid,
            tc.tile_pool(name="stream", bufs=2) as stream,
            tc.tile_pool(name="small", bufs=4) as small,
            tc.tile_pool(name="dram", bufs=1, space="DRAM") as dram,
            ExitStack() as est,
        ):
            # ---- constants ----
            iota_t = constp.tile([128, CPAD], fp32, tag="iota")
            nc.gpsimd.iota(iota_t[:], pattern=[[1, CPAD]], base=0,
                           channel_multiplier=0,
                           allow_small_or_imprecise_dtypes=True)
            ones_row = constp.tile([1, 128], fp8, tag="ones_row")
            nc.vector.memset(ones_row[:], 4.0)
            ident_f = constp.tile([128, 128], fp32, tag="ident_f")
            make_identity(nc, ident_f[:])

            # DRAM bounces
            red_in = dram.tile([CPAD, RW], fp8, tag="red_in")
            a2a_out = dram.tile([CPAD, RW], fp8, tag="a2a_out")
            ag_in = dram.tile([AGR, DBF], fp8, tag="ag_in")
            cn_dram = dram.tile([AGR * NCORES, DBF], fp8, tag="cn",
                                addr_space="Shared")

            f_tiles, lab_tiles = [], []

            with (tc.tile_pool(name="ohp", bufs=1) as ohp,
                  tc.tile_pool(name="psseg", bufs=2, space="PSUM") as psseg):
                # ---- phase 1: load + normalize f (bf16); one-hot (bf16);
                # f tiles carry a trailing ones column for fused counts ----
                oh_tiles, x_tiles = [], []
                for b in range(NB):
                    r0 = b * 128
                    x_t = stream.tile([128, D], bf16, tag="xin", bufs=3,
                                      name=f"x{b}")
                    nc.sync.dma_start(out=x_t[:], in_=feat[r0:r0 + 128, :])
                    x_tiles.append(x_t)
                    lab_t = resid.tile([128, 1], i32, tag=f"lab{b}",
                                       name=f"lab{b}")
                    nc.sync.dma_start(out=lab_t[:], in_=labels[r0:r0 + 128, :])
                    lab_tiles.append(lab_t)
                for b in range(NB):
                    x_t = x_tiles[b]
                    ss = small.tile([128, 1], fp32, tag="ss")
                    scr = stream.tile([128, D], fp32, tag="scrB")
                    nc.scalar.activation(out=scr[:], in_=x_t[:],
                                         func=AF.Square, accum_out=ss[:])
                    nrm = small.tile([128, 1], fp32, tag="nrm")
                    nc.scalar.activation(out=nrm[:], in_=ss[:], func=AF.Sqrt)
                    nc.vector.tensor_scalar_max(nrm[:], nrm[:], 1e-12)
                    rin = small.tile([128, 1], fp32, tag="rin")
                    nc.vector.reciprocal(rin[:], nrm[:])
                    f_t = resid.tile([128, RW], bf16, tag=f"f{b}",
                                     name=f"f{b}")
                    nc.vector.tensor_scalar_mul(f_t[:, 0:D], x_t[:],
                                                rin[:, :1])
                    nc.vector.memset(f_t[:, D:RW], 1.0)
                    f_tiles.append(f_t)
                    lab_f = small.tile([128, 1], fp32, tag="labf")
                    nc.scalar.copy(lab_f[:], lab_tiles[b][:])
                    oh_t = ohp.tile([128, CPAD], bf16, tag=f"oh{b}",
                                    name=f"oh{b}")
                    nc.vector.tensor_scalar(
                        out=oh_t[:], in0=iota_t[:], scalar1=lab_f[:, :1],
                        scalar2=None, op0=ALU.is_equal)
                    oh_tiles.append(oh_t)

                # ---- phase 2: segment sums + fused counts column ----
                for ci in range(NCCH):
                    c0 = ci * 128
                    ps = psseg.tile([128, RW], fp32, tag="segsum", bufs=2)
                    for b in range(NB):
                        st, sp = (b == 0), (b == NB - 1)
                        for n0, nsz in ((0, 512), (512, 512), (1024, 1)):
                            nc.tensor.matmul(
                                ps[:, n0:n0 + nsz],
                                lhsT=oh_tiles[b][:, c0:c0 + 128],
                                rhs=f_tiles[b][:, n0:n0 + nsz],
                                start=st, stop=sp)
                    sums_f = stream.tile([128, RW], fp8, tag="sums_f",
                                         bufs=2)
                    if ci % 2 == 0:
                        nc.scalar.mul(sums_f[:, :], ps[:, :], SSCALE)
                    else:
                        nc.vector.tensor_scalar_mul(sums_f[:, :], ps[:, :],
                                                    SSCALE)
                    nc.sync.dma_start(out=red_in[c0:c0 + 128, :],
                                      in_=sums_f[:, :])

            # ---- phase 3: AllToAll (mesh) -- sender s's chunk k lands on
            # core k rows [128s..128s+128); reduce locally afterwards ----
            nc.gpsimd.collective_compute(
                "AllToAll", ALU.bypass,
                ins=[red_in.opt()], outs=[a2a_out.opt()],
                replica_groups=[list(range(NCORES))])
            psmm = est.enter_context(
                tc.tile_pool(name="psmm", bufs=1, space="PSUM"))

            # ---- comm window: features_adv norms + misc loads ----
            cen = stream.tile([128, D], fp32, tag="scrB")
            nc.sync.dma_start(out=cen[:, :], in_=centers_sh[:, :])
            lg_tiles = []
            for b in range(NB):
                lg_t = resid.tile([128, 1], i32, tag=f"lg{b}",
                                  name=f"lg{b}")
                nc.sync.dma_start(out=lg_t[:], in_=labels_g[:, b:b + 1])
                lg_tiles.append(lg_t)
            xa_tiles = []
            for b in range(NB):
                r0 = b * 128
                xa_t = stream.tile([128, D], bf16, tag="xain", bufs=3)
                nc.sync.dma_start(out=xa_t[:], in_=feat_adv[r0:r0 + 128, :])
                ssa = small.tile([128, 1], fp32, tag="ss")
                scr = stream.tile([128, D], fp32, tag="scrA")
                nc.scalar.activation(out=scr[:], in_=xa_t[:],
                                     func=AF.Square, accum_out=ssa[:])
                nrma = small.tile([128, 1], fp32, tag="nrm")
                nc.scalar.activation(out=nrma[:], in_=ssa[:], func=AF.Sqrt)
                nc.vector.tensor_scalar_max(nrma[:], nrma[:], 1e-12)
                rina = small.tile([128, 1], fp32, tag="rin")
                nc.vector.reciprocal(rina[:], nrma[:])
                xa_bf = resid.tile([128, D], bf16, tag=f"xa{b}",
                                   name=f"xa{b}")
                nc.vector.tensor_scalar_mul(xa_bf[:], xa_t[:], rina[:, :1])
                xa_tiles.append(xa_bf)

            # ---- phase 3b: local reduction of the 8 AllToAll blocks ----
            t8 = stream.tile([128, NCORES * RW], fp8, tag="t8", bufs=1)
            t8o = t8[:, :].rearrange("p (s d) -> p s d", s=NCORES)
            t8v = a2a_out[:, :].rearrange("(s j) d -> j s d", s=NCORES)
            nc.sync.dma_start(out=t8o[:, 0:4, :], in_=t8v[:, 0:4, :])
            nc.scalar.dma_start(out=t8o[:, 4:8, :], in_=t8v[:, 4:8, :])
            p01 = stream.tile([128, RW], fp32, tag="p01", bufs=1)
            nc.vector.tensor_tensor(out=p01[:], in0=t8[:, 0:RW],
                                    in1=t8[:, RW:2 * RW], op=ALU.add)
            p23 = stream.tile([128, RW], fp32, tag="p23", bufs=1)
            nc.vector.tensor_tensor(out=p23[:], in0=t8[:, 2 * RW:3 * RW],
                                    in1=t8[:, 3 * RW:4 * RW], op=ALU.add)
            p45 = stream.tile([128, RW], fp32, tag="p45", bufs=1)
            nc.vector.tensor_tensor(out=p45[:], in0=t8[:, 4 * RW:5 * RW],
                                    in1=t8[:, 5 * RW:6 * RW], op=ALU.add)
            p67 = stream.tile([128, RW], fp32, tag="p67", bufs=1)
            nc.vector.tensor_tensor(out=p67[:], in0=t8[:, 6 * RW:7 * RW],
                                    in1=t8[:, 7 * RW:8 * RW], op=ALU.add)
            nc.vector.tensor_add(p01[:], p01[:], p23[:])
            nc.vector.tensor_add(p45[:], p45[:], p67[:])
            acc = resid.tile([128, RW], fp32, tag="acc")
            nc.vector.tensor_add(acc[:], p01[:], p45[:])

            # ---- phase 4: momentum update of this core's 128 classes ----
            # acc carries 16x sums and 16x counts; the scale cancels in
            # mean = sums16/cnt16, and cnt16>0 iff cnt>0.
            csq_col = small.tile([128, 1], fp32, tag="csq_col")
            cn_bf = resid.tile([128, DBF], fp8, tag="cn_bf")
            cntc = small.tile([128, 1], fp32, tag="cntc")
            nc.vector.tensor_scalar_max(cntc[:], acc[:, D:D + 1], 1.0)
            rcv = small.tile([128, 1], fp32, tag="rcv")
            nc.vector.reciprocal(rcv[:], cntc[:])
            w = small.tile([128, 1], fp32, tag="w")
            nc.vector.tensor_scalar(out=w[:], in0=acc[:, D:D + 1],
                                    scalar1=0.0, scalar2=1.0 - MOM,
                                    op0=ALU.is_gt, op1=ALU.mult)
            m = small.tile([128, 1], fp32, tag="m")
            nc.vector.tensor_tensor(out=m[:], in0=w[:], in1=rcv[:],
                                    op=ALU.mult)
            u = small.tile([128, 1], fp32, tag="u")
            nc.vector.tensor_scalar(out=u[:], in0=w[:], scalar1=-1.0,
                                    scalar2=1.0, op0=ALU.mult, op1=ALU.add)
            t1 = stream.tile([128, D], fp32, tag="scrC")
            nc.scalar.mul(t1[:], cen[:, :], u[:, :1])
            cn_t = stream.tile([128, D], fp32, tag="cn_t")
            nc.vector.scalar_tensor_tensor(
                out=cn_t[:, :], in0=acc[:, 0:D], scalar=m[:, :1],
                in1=t1[:, :], op0=ALU.mult, op1=ALU.add)
            scr2 = stream.tile([128, D], bf16, tag="sqdump")
            nc.scalar.activation(out=scr2[:], in_=cn_t[:, :],
                                 func=AF.Square, accum_out=csq_col[:])
            nc.vector.tensor_copy(cn_bf[:, 0:D], cn_t[:, :])
            nc.vector.tensor_copy(cn_bf[:, D:D + 4].bitcast(fp32),
                                  csq_col[:, :])
            nc.vector.memset(cn_bf[:, D + 4:DBF], 0.0)
            nc.sync.dma_start(out=ag_in[0:128, :], in_=cn_bf[:, :])

            # local transposes of this core's CnT blocks -> ag2 payload
            stage = resid.tile([128, 1024], fp8, tag="stage")
            for dj in range(8):
                tpl = psmm.tile([128, 128], fp32, tag="tpl", bufs=2)
                nc.tensor.transpose(
                    out=tpl[:, :], in_=cn_t[:, dj * 128:(dj + 1) * 128],
                    identity=ident_f[:, :])
                if dj % 2 == 0:
                    nc.scalar.copy(stage[:, dj * 128:(dj + 1) * 128],
                                   tpl[:, :])
                else:
                    nc.vector.tensor_copy(stage[:, dj * 128:(dj + 1) * 128],
                                          tpl[:, :])
            myT = resid.tile([128, 1024], fp8, tag="myT")
            nc.vector.tensor_scalar_mul(myT[:], stage[:, :], -2.0)
            csq_bf = small.tile([1, 128], fp8, tag="csq_bf")
            tpc = psmm.tile([1, 128], fp32, tag="tpc", bufs=1)
            nc.tensor.transpose(out=tpc[:1, :], in_=csq_col[:, :1],
                                identity=ident_f[:, :])
            nc.vector.tensor_scalar(out=csq_bf[:1, :], in0=tpc[:1, :],
                                    scalar1=0.25, scalar2=None,
                                    op0=ALU.mult)
            nc.sync.dma_start(out=ag_in[128:256, 0:1024], in_=stage[:, :])
            nc.sync.dma_start(out=ag_in[256:257, 0:128], in_=csq_bf[:1, :])

            # ---- phase 5: one merged AllGather ----
            nc.gpsimd.collective_compute(
                "AllGather", ALU.bypass,
                ins=[ag_in.opt()], outs=[cn_dram.opt()],
                replica_groups=[list(range(NCORES))])

            # pairwise operands: one contiguous load of all ranks' staged
            # CnT blocks (stg_all[:, r*1024 + dj*128 + j] = CnT row
            # dj*128+d2 of rank r's class j) + the 0.25-csq row.
            rk = cn_dram[:, :].rearrange("(k r) j -> r k j", k=NCORES)
            stg_all = resid.tile([128, NCORES * 1024], fp8, tag="stg_all")
            nc.sync.dma_start(
                out=stg_all[:, :].rearrange("p (k j) -> p k j", k=NCORES),
                in_=rk[128:256, :, 0:1024])
            csq_row = constp.tile([1, 1024], fp8, tag="csq_row")
            nc.scalar.dma_start(out=csq_row[:],
                                in_=rk[256:257, :, 0:128])

            # ---- phase 6: intra losses via fused dots + row csq ----
            dots_f = resid.tile([128, NB], fp32, tag="dots_f")
            dots_a = resid.tile([128, NB], fp32, tag="dots_a")
            csq_gt = resid.tile([128, NB], fp32, tag="csq_gt")
            for b in range(NB):
                g_t = stream.tile([128, D], bf16, tag="gat", bufs=4)
                nc.gpsimd.indirect_dma_start(
                    out=g_t[:], out_offset=None, in_=cn_dram[:, :],
                    in_offset=bass.IndirectOffsetOnAxis(
                        ap=lg_tiles[b][:, :1], axis=0))
                prodf = stream.tile([128, D], bf16, tag="pdump", bufs=3)
                nc.vector.scalar_tensor_tensor(
                    out=prodf[:], in0=f_tiles[b][:, 0:D], scalar=1.0,
                    in1=g_t[:, :], op0=ALU.mult, op1=ALU.mult,
                    accum_out=dots_f[:, b:b + 1])
                proda = stream.tile([128, D], bf16, tag="pdump", bufs=3)
                nc.vector.scalar_tensor_tensor(
                    out=proda[:], in0=xa_tiles[b][:], scalar=1.0,
                    in1=g_t[:, :], op0=ALU.mult, op1=ALU.mult,
                    accum_out=dots_a[:, b:b + 1])
                sqg = stream.tile([128, D], bf16, tag="sqd2", bufs=3)
                nc.scalar.activation(out=sqg[:], in_=g_t[:, :],
                                     func=AF.Square,
                                     accum_out=csq_gt[:, b:b + 1])

            # ---- phase 7: pairwise inter loss (rows = this core's own
            # 128 classes; zero pad rows provably contribute 0) ----
            g_ps = psmm.tile([128, C], fp32, tag="gmm", bufs=1)
            for r in range(NCORES):
                n0 = r * 128
                nsz = min(128, C - n0)
                for dj in range(8):
                    nc.tensor.matmul(
                        g_ps[:, n0:n0 + nsz],
                        lhsT=myT[:, dj * 128:(dj + 1) * 128],
                        rhs=stg_all[:, r * 1024 + dj * 128:
                                    r * 1024 + dj * 128 + nsz],
                        start=(dj == 0), stop=False)
                nc.tensor.matmul(g_ps[:, n0:n0 + nsz],
                                 lhsT=ones_row[:1, :],
                                 rhs=csq_row[:1, n0:n0 + nsz],
                                 start=False, stop=True)

            gq1 = small.tile([128, NB], fp32, tag="gq1")
            nc.vector.tensor_scalar(out=gq1[:], in0=csq_gt[:], scalar1=1.0,
                                    scalar2=None, op0=ALU.add)
            ssf_t = small.tile([128, NB], fp32, tag="ssf_t")
            nc.vector.scalar_tensor_tensor(
                out=ssf_t[:], in0=dots_f[:], scalar=-2.0, in1=gq1[:],
                op0=ALU.mult, op1=ALU.add)
            nc.vector.tensor_scalar_max(ssf_t[:], ssf_t[:], 0.0)
            ssa_t = small.tile([128, NB], fp32, tag="ssa_t")
            nc.vector.scalar_tensor_tensor(
                out=ssa_t[:], in0=dots_a[:], scalar=-2.0, in1=gq1[:],
                op0=ALU.mult, op1=ALU.add)
            nc.vector.tensor_scalar_max(ssa_t[:], ssa_t[:], 0.0)
            dist_f = small.tile([128, NB], fp32, tag="dist_f")
            nc.scalar.activation(out=dist_f[:], in_=ssf_t[:], func=AF.Sqrt)
            dist_a = small.tile([128, NB], fp32, tag="dist_a")
            nc.scalar.activation(out=dist_a[:], in_=ssa_t[:], func=AF.Sqrt)
            ir_f = small.tile([128, 1], fp32, tag="ir_f")
            nc.vector.tensor_reduce(out=ir_f[:], in_=dist_f[:], axis=AX.X,
                                    op=ALU.add)
            ir_a = small.tile([128, 1], fp32, tag="ir_a")
            nc.vector.tensor_reduce(out=ir_a[:], in_=dist_a[:], axis=AX.X,
                                    op=ALU.add)
            intra_rows = small.tile([128, 1], fp32, tag="intra_rows")
            nc.vector.tensor_add(intra_rows[:], ir_f[:], ir_a[:])

            d2b = stream.tile([128, C], fp32, tag="scrB")
            nc.vector.tensor_scalar(
                out=d2b[:], in0=g_ps[:, :],
                scalar1=csq_col[:, :1],
                scalar2=0.0, op0=ALU.add, op1=ALU.max)
            dst = stream.tile([128, C], fp32, tag="scrC")
            nc.scalar.activation(out=dst[:], in_=d2b[:], func=AF.Sqrt)
            term = stream.tile([128, C], fp32, tag="scrA")
            inter_rows = small.tile([128, 1], fp32, tag="inter_rows")
            nc.scalar.activation(out=term[:], in_=dst[:],
                                 func=AF.Relu, bias=1.0, scale=-1.0,
                                 accum_out=inter_rows[:])

            # ---- phase 8: final reduce + partials out ----
            partials = small.tile([128, 2], fp32, tag="partials")
            nc.vector.memset(partials[:], 0.0)
            nc.vector.tensor_copy(partials[:, 0:1], intra_rows[:])
            nc.vector.tensor_copy(partials[:, 1:2], inter_rows[:, :])
            pr = small.tile([1, 2], fp32, tag="pr")
            nc.gpsimd.tensor_reduce(out=pr[:1, :], in_=partials[:, :],
                                    axis=AX.C, op=ALU.add)
            nc.sync.dma_start(out=out[0:1, 0:2], in_=pr[:1, :])

    nc.compile()
    return nc


def _get_nc():
    if "nc" not in _state:
        _state["nc"] = _build()
    return _state["nc"]


def kernel(features, features_adv, centers, labels):
    from concourse import bass_utils

    nc = _get_nc()
    import ml_dtypes
    features_bf = np.ascontiguousarray(
        np.asarray(features, dtype=np.float32).astype(ml_dtypes.bfloat16))
    features_adv_bf = np.ascontiguousarray(
        np.asarray(features_adv, dtype=np.float32).astype(ml_dtypes.bfloat16))
    centers_np = np.asarray(centers, dtype=np.float32)
    centers_pad = np.zeros((CPAD, D), dtype=np.float32)
    centers_pad[:C] = centers_np
    labels_i32 = np.ascontiguousarray(
        np.asarray(labels).astype(np.int32).reshape(B, 1))
    # gather-row remap: class c lives at AG row 257*(c//128) + c%128
    labels_gr = (AGR * (labels_i32 >> 7) + (labels_i32 & 127)).astype(
        np.int32).reshape(B)

    in_maps = []
    for k in range(NCORES):
        sl = slice(k * BLOC, (k + 1) * BLOC)

        in_maps.append({
            "features": features_bf[sl],
            "features_adv": features_adv_bf[sl],
            "centers_sh": np.ascontiguousarray(
                centers_pad[k * 128:(k + 1) * 128]),
            "labels": labels_i32[sl],
            "labels_g": np.ascontiguousarray(
                labels_gr[sl].reshape(NB, 128).T),
        })

    res = bass_utils.run_bass_kernel_spmd(
        nc, in_maps, core_ids=list(range(NCORES)),
        trace=bool(int(os.environ.get("AFD_TRACE", "0"))))
    _state["last_results"] = res
    parts = np.stack([res.results[k]["out"][0] for k in range(NCORES)])
    intra_sum = float(parts[:, 0].sum())
    inter_sum = float(parts[:, 1].sum())
    val = intra_sum / B - 0.25 * (inter_sum - C) / N_PAIRS
    return np.asarray(np.float32(val))


# revision 4
# speedup vs baseline: 1.5931x; 1.5931x over previous
"""AFD loss kernel for 8 TRN2 NeuronCores (Bass/Tile).

Algorithm (matches the reference loss_fn):
  f  = l2norm(features); fa = l2norm(features_adv)
  per-class sums/counts of f via one-hot matmul
  centers_new = where(counts>0, 0.9*centers + 0.1*sums/max(counts,1), centers)
  intra = mean ||f - centers_new[labels]|| + mean ||fa - centers_new[labels]||
        with ||x - c||^2 = ||x||^2 - 2 x.c + ||c||^2    (x unit-norm)
  inter = sum_{i<j} relu(1 - ||ci - cj||) / n_pairs   (symmetric full-sum trick)
  loss  = intra - 0.5 * inter

Structure (v14, label-sharded):
  - the batch is sharded BY LABEL OWNERSHIP on the host: core k receives
    exactly the samples whose label is in [128k, 128k+128), padded with
    zero rows (one-hot label -1 -> all-zero row; masked out of intra).
    Segment sums/counts are then fully LOCAL - no cross-core reduction
    collective at all.  The momentum update runs locally in exact fp32.
  - one-hot matmul shrinks to [128 batch x 128 local classes] per tile
    (~8x less PE work than all-class segment sums); the count column is
    fused as column 1024 of the f tiles (ones column)
  - intra: updated center rows (bf16) + exact fp32 csq (bitcast into two
    bf16 columns) are written to a LOCAL DRAM row buffer; per-tile
    indirect gathers + DVE products (2x mode) + ACT accumulation; per-row
    validity mask applied after the sqrt
  - the ONLY collective is a small fp8 AllGather (129 rows/rank: locally
    PE-transposed CnT blocks + 0.25x-scaled csq row, recovered by a
    4.0-valued ones-row matmul) feeding the pairwise inter block
  - inter: per-rank column blocks of -2*Cn_my @ Cn.T from one contiguous
    stage load; zero pad rows/cols provably contribute 0; symmetric
    full-sum with the C diagonal terms removed in the final formula
  - per-core [intra_sum, inter_sum] partials; host sums 8x2 floats and
    applies the affine formula (the unshard step)
"""

import os
from contextlib import ExitStack

import numpy as np

NCORES = 8
B = 8192
D = 1024
C = 1000
MOM = 0.9
N_PAIRS = C * (C - 1) / 2.0
CPAD = 1024                 # classes padded to full chunks
GW = D + 4                  # gather row: D bf16 + csq (f32 as 2 bf16) + pad
SW = D + 8                  # AG stage row width (fp8)
AGR = 129                   # AG rows/rank: 128 cnT-stage + 1 csq
RW = D + 1                  # reduce row width: sums + count column

_state = {}


def _build(nbt):
    import concourse.bacc as bacc
    import concourse.bass as bass
    import concourse.mybir as mybir
    import concourse.tile as tile
    from concourse.masks import make_identity

    fp32 = mybir.dt.float32
    bf16 = mybir.dt.bfloat16
    fp8 = mybir.dt.float8e4
    i32 = mybir.dt.int32
    AF = mybir.ActivationFunctionType
    ALU = mybir.AluOpType
    AX = mybir.AxisListType

    bpc = nbt * 128

    nc = bacc.Bacc("TRN2", target_bir_lowering=False, debug=False,
                   num_devices=NCORES)

    feat = nc.dram_tensor("features", [bpc, D], bf16, kind="ExternalInput")
    feat_adv = nc.dram_tensor("features_adv", [bpc, D], bf16,
                              kind="ExternalInput")
    centers_sh = nc.dram_tensor("centers_sh", [128, D], fp32,
                                kind="ExternalInput")
    labels = nc.dram_tensor("labels", [bpc, 1], i32, kind="ExternalInput")
    labels_g = nc.dram_tensor("labels_g", [128, nbt], i32,
                              kind="ExternalInput")
    out = nc.dram_tensor("out", [1, 2], fp32, kind="ExternalOutput")

    with tile.TileContext(nc) as tc:
        with (
            tc.tile_pool(name="const", bufs=1) as constp,
            tc.tile_pool(name="resid", bufs=1) as resid,
            tc.tile_pool(name="stream", bufs=2) as stream,
            tc.tile_pool(name="small", bufs=4) as small,
            tc.tile_pool(name="psall", bufs=1, space="PSUM") as psall,
            tc.tile_pool(name="dram", bufs=1, space="DRAM") as dram,
        ):
            # ---- constants ----
            iota_t = constp.tile([128, 128], fp32, tag="iota")
            nc.gpsimd.iota(iota_t[:], pattern=[[1, 128]], base=0,
                           channel_multiplier=0,
                           allow_small_or_imprecise_dtypes=True)
            ones_row = constp.tile([1, 128], fp8, tag="ones_row")
            nc.vector.memset(ones_row[:], 4.0)
            ident_f = constp.tile([128, 128], fp32, tag="ident_f")
            make_identity(nc, ident_f[:])

            # DRAM bounces
            gbuf = dram.tile([128, GW], bf16, tag="gbuf")
            ag_in = dram.tile([AGR, SW], fp8, tag="ag_in")
            cn_dram = dram.tile([AGR * NCORES, SW], fp8, tag="cn",
                                addr_space="Shared")

            # ---- phase 1: load + normalize f; local one-hot; mask ----
            f_tiles, lab_tiles, x_tiles = [], [], []
            mask_nb = resid.tile([128, nbt], fp32, tag="mask_nb")
            for b in range(nbt):
                r0 = b * 128
                x_t = stream.tile([128, D], bf16, tag="xin", bufs=3,
                                  name=f"x{b}")
                nc.sync.dma_start(out=x_t[:], in_=feat[r0:r0 + 128, :])
                x_tiles.append(x_t)
                lab_t = resid.tile([128, 1], i32, tag=f"lab{b}",
                                   name=f"lab{b}")
                nc.sync.dma_start(out=lab_t[:], in_=labels[r0:r0 + 128, :])
                lab_tiles.append(lab_t)
            cen = resid.tile([128, D], fp32, tag="cen")
            nc.sync.dma_start(out=cen[:, :], in_=centers_sh[:, :])
            lg_tiles = []
            for b in range(nbt):
                lg_t = resid.tile([128, 1], i32, tag=f"lg{b}",
                                  name=f"lg{b}")
                nc.sync.dma_start(out=lg_t[:], in_=labels_g[:, b:b + 1])
                lg_tiles.append(lg_t)

            oh_tiles = []
            for b in range(nbt):
                x_t = x_tiles[b]
                ss = small.tile([128, 1], fp32, tag="ss")
                scr = stream.tile([128, D], fp32, tag="scrB")
                nc.scalar.activation(out=scr[:], in_=x_t[:],
                                     func=AF.Square, accum_out=ss[:])
                nrm = small.tile([128, 1], fp32, tag="nrm")
                nc.scalar.activation(out=nrm[:], in_=ss[:], func=AF.Sqrt)
                nc.vector.tensor_scalar_max(nrm[:], nrm[:], 1e-12)
                rin = small.tile([128, 1], fp32, tag="rin")
                nc.vector.reciprocal(rin[:], nrm[:])
                f_t = resid.tile([128, RW], bf16, tag=f"f{b}",
                                 name=f"f{b}")
                nc.vector.tensor_scalar_mul(f_t[:, 0:D], x_t[:],
                                            rin[:, :1])
                nc.vector.memset(f_t[:, D:RW], 1.0)
                f_tiles.append(f_t)
                lab_f = small.tile([128, 1], fp32, tag="labf")
                nc.scalar.copy(lab_f[:], lab_tiles[b][:])
                nc.vector.tensor_scalar(
                    out=mask_nb[:, b:b + 1], in0=lab_f[:], scalar1=-0.5,
                    scalar2=None, op0=ALU.is_gt)
                oh_t = resid.tile([128, 128], bf16, tag=f"oh{b}",
                                  name=f"oh{b}")
                nc.vector.tensor_scalar(
                    out=oh_t[:], in0=iota_t[:], scalar1=lab_f[:, :1],
                    scalar2=None, op0=ALU.is_equal)
                oh_tiles.append(oh_t)

            # ---- phase 2: local segment sums + fused counts column ----
            ps = psall.tile([128, RW], fp32, tag="segsum", bufs=1)
            for b in range(nbt):
                st, sp = (b == 0), (b == nbt - 1)
                for n0, nsz in ((0, 512), (512, 512), (1024, 1)):
                    nc.tensor.matmul(
                        ps[:, n0:n0 + nsz],
                        lhsT=oh_tiles[b][:, :],
                        rhs=f_tiles[b][:, n0:n0 + nsz],
                        start=st, stop=sp)

            # ---- phase 3: momentum update (local, exact fp32) ----
            csq_col = small.tile([128, 1], fp32, tag="csq_col")
            cntc = small.tile([128, 1], fp32, tag="cntc")
            nc.vector.tensor_scalar_max(cntc[:], ps[:, D:D + 1], 1.0)
            rcv = small.tile([128, 1], fp32, tag="rcv")
            nc.vector.reciprocal(rcv[:], cntc[:])
            w = small.tile([128, 1], fp32, tag="w")
            nc.vector.tensor_scalar(out=w[:], in0=ps[:, D:D + 1],
                                    scalar1=0.0, scalar2=1.0 - MOM,
                                    op0=ALU.is_gt, op1=ALU.mult)
            m = small.tile([128, 1], fp32, tag="m")
            nc.vector.tensor_tensor(out=m[:], in0=w[:], in1=rcv[:],
                                    op=ALU.mult)
            u = small.tile([128, 1], fp32, tag="u")
            nc.vector.tensor_scalar(out=u[:], in0=w[:], scalar1=-1.0,
                                    scalar2=1.0, op0=ALU.mult, op1=ALU.add)
            t1 = stream.tile([128, D], fp32, tag="scrC")
            nc.scalar.mul(t1[:], cen[:, :], u[:, :1])
            cn_t = resid.tile([128, D], fp32, tag="cn_t")
            nc.vector.scalar_tensor_tensor(
                out=cn_t[:, :], in0=ps[:, 0:D], scalar=m[:, :1],
                in1=t1[:, :], op0=ALU.mult, op1=ALU.add)
            scr2 = stream.tile([128, D], bf16, tag="sqdump")
            nc.scalar.activation(out=scr2[:], in_=cn_t[:, :],
                                 func=AF.Square, accum_out=csq_col[:])
            # local gather-row buffer: bf16 rows + exact fp32 csq bitcast
            cnr = resid.tile([128, GW], bf16, tag="cnr")
            nc.vector.tensor_copy(cnr[:, 0:D], cn_t[:, :])
            nc.vector.tensor_copy(cnr[:, D:D + 2].bitcast(fp32),
                                  csq_col[:, :])
            nc.vector.memset(cnr[:, D + 2:GW], 0.0)
            nc.sync.dma_start(out=gbuf[:, :], in_=cnr[:, :])

            # local transposes of this core's CnT blocks -> AG payload
            stage = resid.tile([128, 1024], fp8, tag="stage")
            for dj in range(8):
                tpl = psall.tile([128, 128], fp32, tag="tpl", bufs=2)
                nc.tensor.transpose(
                    out=tpl[:, :], in_=cn_t[:, dj * 128:(dj + 1) * 128],
                    identity=ident_f[:, :])
                if dj % 2 == 0:
                    nc.scalar.copy(stage[:, dj * 128:(dj + 1) * 128],
                                   tpl[:, :])
                else:
                    nc.vector.tensor_copy(stage[:, dj * 128:(dj + 1) * 128],
                                          tpl[:, :])
            myT = resid.tile([128, 1024], fp8, tag="myT")
            nc.vector.tensor_scalar_mul(myT[:], stage[:, :], -2.0)
            csq_bf = small.tile([1, 128], fp8, tag="csq_bf")
            tpc = psall.tile([1, 128], fp32, tag="tpc", bufs=1)
            nc.tensor.transpose(out=tpc[:1, :], in_=csq_col[:, :1],
                                identity=ident_f[:, :])
            nc.vector.tensor_scalar(out=csq_bf[:1, :], in0=tpc[:1, :],
                                    scalar1=0.25, scalar2=None,
                                    op0=ALU.mult)
            nc.sync.dma_start(out=ag_in[0:128, 0:1024], in_=stage[:, :])
            nc.sync.dma_start(out=ag_in[128:129, 0:128], in_=csq_bf[:1, :])

            # ---- phase 4: the only collective - small fp8 AllGather ----
            nc.gpsimd.collective_compute(
                "AllGather", ALU.bypass,
                ins=[ag_in.opt()], outs=[cn_dram.opt()],
                replica_groups=[list(range(NCORES))])

            # ---- comm window: features_adv norms ----
            xa_tiles = []
            for b in range(nbt):
                r0 = b * 128
                xa_t = stream.tile([128, D], bf16, tag="xain", bufs=3)
                nc.sync.dma_start(out=xa_t[:], in_=feat_adv[r0:r0 + 128, :])
                ssa = small.tile([128, 1], fp32, tag="ss")
                scr = stream.tile([128, D], fp32, tag="scrA")
                nc.scalar.activation(out=scr[:], in_=xa_t[:],
                                     func=AF.Square, accum_out=ssa[:])
                nrma = small.tile([128, 1], fp32, tag="nrm")
                nc.scalar.activation(out=nrma[:], in_=ssa[:], func=AF.Sqrt)
                nc.vector.tensor_scalar_max(nrma[:], nrma[:], 1e-12)
                rina = small.tile([128, 1], fp32, tag="rin")
                nc.vector.reciprocal(rina[:], nrma[:])
                xa_bf = resid.tile([128, D], bf16, tag=f"xa{b}",
                                   name=f"xa{b}")
                nc.vector.tensor_scalar_mul(xa_bf[:], xa_t[:], rina[:, :1])
                xa_tiles.append(xa_bf)

            # ---- phase 5: intra dots via LOCAL gathers (no AG dep) ----
            dots_f = resid.tile([128, nbt], fp32, tag="dots_f")
            dots_a = resid.tile([128, nbt], fp32, tag="dots_a")
            csqg = resid.tile([128, nbt], fp32, tag="csqg")
            for b in range(nbt):
                g_t = stream.tile([128, GW], bf16, tag="gat", bufs=4)
                nc.gpsimd.indirect_dma_start(
                    out=g_t[:], out_offset=None, in_=gbuf[:, :],
                    in_offset=bass.IndirectOffsetOnAxis(
                        ap=lg_tiles[b][:, :1], axis=0))
                prodf = stream.tile([128, D], bf16, tag="pdump", bufs=3)
                nc.vector.tensor_tensor(out=prodf[:], in0=f_tiles[b][:, 0:D],
                                        in1=g_t[:, 0:D], op=ALU.mult)
                dmp = stream.tile([128, D], bf16, tag="adump", bufs=3)
                nc.scalar.activation(out=dmp[:], in_=prodf[:], func=AF.Copy,
                                     accum_out=dots_f[:, b:b + 1])
                proda = stream.tile([128, D], bf16, tag="pdump", bufs=3)
                nc.vector.tensor_tensor(out=proda[:], in0=xa_tiles[b][:],
                                        in1=g_t[:, 0:D], op=ALU.mult)
                dmp2 = stream.tile([128, D], bf16, tag="adump", bufs=3)
                nc.scalar.activation(out=dmp2[:], in_=proda[:], func=AF.Copy,
                                     accum_out=dots_a[:, b:b + 1])
                nc.vector.tensor_copy(csqg[:, b:b + 1],
                                      g_t[:, D:D + 2].bitcast(fp32))

            # intra finalize: d^2 = mask - 2 dot + csq_g; mask the dist
            base = small.tile([128, nbt], fp32, tag="base")
            nc.vector.tensor_add(base[:], csqg[:], mask_nb[:])
            ssf_t = small.tile([128, nbt], fp32, tag="ssf_t")
            nc.vector.scalar_tensor_tensor(
                out=ssf_t[:], in0=dots_f[:], scalar=-2.0, in1=base[:],
                op0=ALU.mult, op1=ALU.add)
            nc.vector.tensor_scalar_max(ssf_t[:], ssf_t[:], 0.0)
            ssa_t = small.tile([128, nbt], fp32, tag="ssa_t")
            nc.vector.scalar_tensor_tensor(
                out=ssa_t[:], in0=dots_a[:], scalar=-2.0, in1=base[:],
                op0=ALU.mult, op1=ALU.add)
            nc.vector.tensor_scalar_max(ssa_t[:], ssa_t[:], 0.0)
            dist_f = small.tile([128, nbt], fp32, tag="dist_f")
            nc.scalar.activation(out=dist_f[:], in_=ssf_t[:], func=AF.Sqrt)
            dist_a = small.tile([128, nbt], fp32, tag="dist_a")
            nc.scalar.activation(out=dist_a[:], in_=ssa_t[:], func=AF.Sqrt)
            nc.vector.tensor_mul(dist_f[:], dist_f[:], mask_nb[:])
            nc.vector.tensor_mul(dist_a[:], dist_a[:], mask_nb[:])
            ir_f = small.tile([128, 1], fp32, tag="ir_f")
            nc.vector.tensor_reduce(out=ir_f[:], in_=dist_f[:], axis=AX.X,
                                    op=ALU.add)
            ir_a = small.tile([128, 1], fp32, tag="ir_a")
            nc.vector.tensor_reduce(out=ir_a[:], in_=dist_a[:], axis=AX.X,
                                    op=ALU.add)
            intra_rows = small.tile([128, 1], fp32, tag="intra_rows")
            nc.vector.tensor_add(intra_rows[:], ir_f[:], ir_a[:])

            # ---- phase 6: pairwise inter from the AllGather ----
            rk = cn_dram[:, :].rearrange("(k r) j -> r k j", k=NCORES)
            stg_all = resid.tile([128, NCORES * 1024], fp8, tag="stg_all")
            nc.sync.dma_start(
                out=stg_all[:, :].rearrange("p (k j) -> p k j", k=NCORES),
                in_=rk[0:128, :, 0:1024])
            csq_row = constp.tile([1, 1024], fp8, tag="csq_row")
            nc.gpsimd.dma_start(out=csq_row[:],
                                in_=rk[128:129, :, 0:128])

            g_ps = psall.tile([128, C], fp32, tag="gmm", bufs=1)
            for r in range(NCORES):
                n0 = r * 128
                nsz = min(128, C - n0)
                for dj in range(8):
                    nc.tensor.matmul(
                        g_ps[:, n0:n0 + nsz],
                        lhsT=myT[:, dj * 128:(dj + 1) * 128],
                        rhs=stg_all[:, r * 1024 + dj * 128:
                                    r * 1024 + dj * 128 + nsz],
                        start=(dj == 0), stop=False)
                nc.tensor.matmul(g_ps[:, n0:n0 + nsz],
                                 lhsT=ones_row[:1, :],
                                 rhs=csq_row[:1, n0:n0 + nsz],
                                 start=False, stop=True)

            d2b = stream.tile([128, C], fp32, tag="scrB")
            nc.vector.tensor_scalar(
                out=d2b[:], in0=g_ps[:, :],
                scalar1=csq_col[:, :1],
                scalar2=0.0, op0=ALU.add, op1=ALU.max)
            dst = stream.tile([128, C], fp32, tag="scrC")
            nc.scalar.activation(out=dst[:], in_=d2b[:], func=AF.Sqrt)
            term = stream.tile([128, C], fp32, tag="scrA")
            inter_rows = small.tile([128, 1], fp32, tag="inter_rows")
            nc.scalar.activation(out=term[:], in_=dst[:],
                                 func=AF.Relu, bias=1.0, scale=-1.0,
                                 accum_out=inter_rows[:])

            # ---- phase 7: final reduce + partials out ----
            partials = small.tile([128, 2], fp32, tag="partials")
            nc.vector.memset(partials[:], 0.0)
            nc.vector.tensor_copy(partials[:, 0:1], intra_rows[:])
            nc.vector.tensor_copy(partials[:, 1:2], inter_rows[:, :])
            pr = small.tile([1, 2], fp32, tag="pr")
            nc.gpsimd.tensor_reduce(out=pr[:1, :], in_=partials[:, :],
                                    axis=AX.C, op=ALU.add)
            nc.sync.dma_start(out=out[0:1, 0:2], in_=pr[:1, :])

    nc.compile()
    return nc


def _get_nc(nbt):
    key = ("nc", nbt)
    if key not in _state:
        _state[key] = _build(nbt)
    return _state[key]


def kernel(features, features_adv, centers, labels):
    from concourse import bass_utils
    import ml_dtypes

    labels_np = np.asarray(labels).astype(np.int64).reshape(-1)
    own = (labels_np >> 7).astype(np.int64)
    counts = np.bincount(own, minlength=NCORES)
    nbt = int(np.ceil(max(int(counts.max()), 1) / 128.0))
    bpc = nbt * 128
    nc = _get_nc(nbt)

    features_bf = np.asarray(features, dtype=np.float32).astype(
        ml_dtypes.bfloat16)
    features_adv_bf = np.asarray(features_adv, dtype=np.float32).astype(
        ml_dtypes.bfloat16)
    centers_np = np.asarray(centers, dtype=np.float32)
    centers_pad = np.zeros((CPAD, D), dtype=np.float32)
    centers_pad[:C] = centers_np

    in_maps = []
    for k in range(NCORES):
        idx = np.nonzero(own == k)[0]
        nk = len(idx)
        fk = np.zeros((bpc, D), dtype=ml_dtypes.bfloat16)
        fk[:nk] = features_bf[idx]
        fak = np.zeros((bpc, D), dtype=ml_dtypes.bfloat16)
        fak[:nk] = features_adv_bf[idx]
        loc = (labels_np[idx] - 128 * k).astype(np.int32)
        labk = np.full((bpc, 1), -1, dtype=np.int32)
        labk[:nk, 0] = loc
        lgk = np.zeros((bpc,), dtype=np.int32)
        lgk[:nk] = loc
        in_maps.append({
            "features": fk,
            "features_adv": fak,
            "centers_sh": np.ascontiguousarray(
                centers_pad[k * 128:(k + 1) * 128]),
            "labels": labk,
            "labels_g": np.ascontiguousarray(lgk.reshape(nbt, 128).T),
        })

    res = bass_utils.run_bass_kernel_spmd(
        nc, in_maps, core_ids=list(range(NCORES)),
        trace=bool(int(os.environ.get("AFD_TRACE", "0"))))
    _state["last_results"] = res
    parts = np.stack([res.results[k]["out"][0] for k in range(NCORES)])
    intra_sum = float(parts[:, 0].sum())
    inter_sum = float(parts[:, 1].sum())
    val = intra_sum / B - 0.25 * (inter_sum - C) / N_PAIRS
    return np.asarray(np.float32(val))


# revision 8
# speedup vs baseline: 2.0115x; 1.2626x over previous
"""AFD loss kernel for 8 TRN2 NeuronCores (Bass/Tile).

Algorithm (matches the reference loss_fn):
  f  = l2norm(features); fa = l2norm(features_adv)
  per-class sums/counts of f via one-hot matmul
  centers_new = where(counts>0, 0.9*centers + 0.1*sums/max(counts,1), centers)
  intra = mean ||f - centers_new[labels]|| + mean ||fa - centers_new[labels]||
        with ||x - c||^2 = ||x||^2 - 2 x.c + ||c||^2    (x unit-norm)
  inter = sum_{i<j} relu(1 - ||ci - cj||) / n_pairs   (symmetric full-sum trick)
  loss  = intra - 0.5 * inter

Structure (v14, label-sharded):
  - the batch is sharded BY LABEL OWNERSHIP on the host: core k receives
    exactly the samples whose label is in [128k, 128k+128), padded with
    zero rows (one-hot label -1 -> all-zero row; masked out of intra).
    Segment sums/counts are then fully LOCAL - no cross-core reduction
    collective at all.  The momentum update runs locally in exact fp32.
  - one-hot matmul shrinks to [128 batch x 128 local classes] per tile
    (~8x less PE work than all-class segment sums); the count column is
    fused as column 1024 of the f tiles (ones column)
  - intra: updated center rows (bf16) + exact fp32 csq (bitcast into two
    bf16 columns) are written to a LOCAL DRAM row buffer; per-tile
    indirect gathers + DVE products (2x mode) + ACT accumulation; per-row
    validity mask applied after the sqrt
  - the ONLY collective is a small fp8 AllGather (129 rows/rank: locally
    PE-transposed CnT blocks + 0.25x-scaled csq row, recovered by a
    4.0-valued ones-row matmul) feeding the pairwise inter block
  - inter: per-rank column blocks of -2*Cn_my @ Cn.T from one contiguous
    stage load; zero pad rows/cols provably contribute 0; symmetric
    full-sum with the C diagonal terms removed in the final formula
  - per-core [intra_sum, inter_sum] partials; host sums 8x2 floats and
    applies the affine formula (the unshard step)
"""

import os
from contextlib import ExitStack

import numpy as np

NCORES = 8
B = 8192
D = 1024
C = 1000
MOM = 0.9
N_PAIRS = C * (C - 1) / 2.0
CPAD = 1024                 # classes padded to full chunks
GW = D + 4                  # gather row: D bf16 + csq (f32 as 2 bf16) + pad
SW = D + 8                  # AG stage row width (fp8)
AGR = 129                   # AG rows/rank: 128 cnT-stage + 1 csq
RW = D + 1                  # reduce row width: sums + count column

_state = {}


def _build(nbt):
    import concourse.bacc as bacc
    import concourse.bass as bass
    import concourse.mybir as mybir
    import concourse.tile as tile
    from concourse.masks import make_identity

    fp32 = mybir.dt.float32
    bf16 = mybir.dt.bfloat16
    fp8 = mybir.dt.float8e4
    i32 = mybir.dt.int32
    AF = mybir.ActivationFunctionType
    ALU = mybir.AluOpType
    AX = mybir.AxisListType

    bpc = nbt * 128

    nc = bacc.Bacc("TRN2", target_bir_lowering=False, debug=False,
                   num_devices=NCORES)

    feat = nc.dram_tensor("features", [bpc, D], bf16, kind="ExternalInput")
    feat_adv = nc.dram_tensor("features_adv", [bpc, D], bf16,
                              kind="ExternalInput")
    centers_sh = nc.dram_tensor("centers_sh", [128, D], fp32,
                                kind="ExternalInput")
    labels = nc.dram_tensor("labels", [bpc, 1], i32, kind="ExternalInput")
    labels_g = nc.dram_tensor("labels_g", [128, nbt], i32,
                              kind="ExternalInput")
    out = nc.dram_tensor("out", [1, 2], fp32, kind="ExternalOutput")

    with tile.TileContext(nc) as tc:
        with (
            tc.tile_pool(name="const", bufs=1) as constp,
            tc.tile_pool(name="resid", bufs=1) as resid,
            tc.tile_pool(name="stream", bufs=2) as stream,
            tc.tile_pool(name="small", bufs=4) as small,
            tc.tile_pool(name="psall", bufs=1, space="PSUM") as psall,
            tc.tile_pool(name="dram", bufs=1, space="DRAM") as dram,
        ):
            # ---- constants ----
            iota_t = constp.tile([128, 128], fp32, tag="iota")
            nc.gpsimd.iota(iota_t[:], pattern=[[1, 128]], base=0,
                           channel_multiplier=0,
                           allow_small_or_imprecise_dtypes=True)
            ones_row = constp.tile([1, 128], fp8, tag="ones_row")
            nc.vector.memset(ones_row[:], 4.0)
            ident_f = constp.tile([128, 128], fp32, tag="ident_f")
            make_identity(nc, ident_f[:])

            # DRAM bounces
            gbuf = dram.tile([128, GW], bf16, tag="gbuf")
            ag_in = dram.tile([AGR, SW], fp8, tag="ag_in")
            cn_dram = dram.tile([AGR * NCORES, SW], fp8, tag="cn",
                                addr_space="Shared")

            # ---- phase 1: load + normalize f; local one-hot; mask ----
            f_tiles, lab_tiles, x_tiles = [], [], []
            mask_nb = resid.tile([128, nbt], fp32, tag="mask_nb")
            for b in range(nbt):
                r0 = b * 128
                x_t = stream.tile([128, D], bf16, tag="xin", bufs=3,
                                  name=f"x{b}")
                nc.sync.dma_start(out=x_t[:], in_=feat[r0:r0 + 128, :])
                x_tiles.append(x_t)
                lab_t = resid.tile([128, 1], i32, tag=f"lab{b}",
                                   name=f"lab{b}")
                nc.sync.dma_start(out=lab_t[:], in_=labels[r0:r0 + 128, :])
                lab_tiles.append(lab_t)
            cen = resid.tile([128, D], fp32, tag="cen")
            nc.sync.dma_start(out=cen[:, :], in_=centers_sh[:, :])
            lg_tiles = []
            for b in range(nbt):
                lg_t = resid.tile([128, 1], i32, tag=f"lg{b}",
                                  name=f"lg{b}")
                nc.sync.dma_start(out=lg_t[:], in_=labels_g[:, b:b + 1])
                lg_tiles.append(lg_t)

            oh_tiles = []
            for b in range(nbt):
                x_t = x_tiles[b]
                ss = small.tile([128, 1], fp32, tag="ss")
                scr = stream.tile([128, D], fp32, tag="scrB")
                nc.scalar.activation(out=scr[:], in_=x_t[:],
                                     func=AF.Square, accum_out=ss[:])
                nrm = small.tile([128, 1], fp32, tag="nrm")
                nc.scalar.activation(out=nrm[:], in_=ss[:], func=AF.Sqrt)
                nc.vector.tensor_scalar_max(nrm[:], nrm[:], 1e-12)
                rin = small.tile([128, 1], fp32, tag="rin")
                nc.vector.reciprocal(rin[:], nrm[:])
                f_t = resid.tile([128, RW], bf16, tag=f"f{b}",
                                 name=f"f{b}")
                nc.vector.tensor_scalar_mul(f_t[:, 0:D], x_t[:],
                                            rin[:, :1])
                nc.vector.memset(f_t[:, D:RW], 1.0)
                f_tiles.append(f_t)
                lab_f = small.tile([128, 1], fp32, tag="labf")
                nc.scalar.copy(lab_f[:], lab_tiles[b][:])
                nc.vector.tensor_scalar(
                    out=mask_nb[:, b:b + 1], in0=lab_f[:], scalar1=-0.5,
                    scalar2=None, op0=ALU.is_gt)
                oh_t = resid.tile([128, 128], bf16, tag=f"oh{b}",
                                  name=f"oh{b}")
                nc.vector.tensor_scalar(
                    out=oh_t[:], in0=iota_t[:], scalar1=lab_f[:, :1],
                    scalar2=None, op0=ALU.is_equal)
                oh_tiles.append(oh_t)

            # ---- phase 2: local segment sums + fused counts column ----
            ps = psall.tile([128, RW], fp32, tag="segsum", bufs=1)
            for b in range(nbt):
                st, sp = (b == 0), (b == nbt - 1)
                for n0, nsz in ((0, 512), (512, 512), (1024, 1)):
                    nc.tensor.matmul(
                        ps[:, n0:n0 + nsz],
                        lhsT=oh_tiles[b][:, :],
                        rhs=f_tiles[b][:, n0:n0 + nsz],
                        start=st, stop=sp)

            # ---- phase 3: momentum update (local, exact fp32) ----
            csq_col = small.tile([128, 1], fp32, tag="csq_col")
            cntc = small.tile([128, 1], fp32, tag="cntc")
            nc.vector.tensor_scalar_max(cntc[:], ps[:, D:D + 1], 1.0)
            rcv = small.tile([128, 1], fp32, tag="rcv")
            nc.vector.reciprocal(rcv[:], cntc[:])
            w = small.tile([128, 1], fp32, tag="w")
            nc.vector.tensor_scalar(out=w[:], in0=ps[:, D:D + 1],
                                    scalar1=0.0, scalar2=1.0 - MOM,
                                    op0=ALU.is_gt, op1=ALU.mult)
            m = small.tile([128, 1], fp32, tag="m")
            nc.vector.tensor_tensor(out=m[:], in0=w[:], in1=rcv[:],
                                    op=ALU.mult)
            u = small.tile([128, 1], fp32, tag="u")
            nc.vector.tensor_scalar(out=u[:], in0=w[:], scalar1=-1.0,
                                    scalar2=1.0, op0=ALU.mult, op1=ALU.add)
            t1 = stream.tile([128, D], fp32, tag="scrC")
            nc.scalar.mul(t1[:], cen[:, :], u[:, :1])
            cn_t = resid.tile([128, D], fp32, tag="cn_t")
            nc.vector.scalar_tensor_tensor(
                out=cn_t[:, :], in0=ps[:, 0:D], scalar=m[:, :1],
                in1=t1[:, :], op0=ALU.mult, op1=ALU.add)
            scr2 = stream.tile([128, D], bf16, tag="sqdump")
            nc.scalar.activation(out=scr2[:], in_=cn_t[:, :],
                                 func=AF.Square, accum_out=csq_col[:])
            # local gather-row buffer: bf16 rows + exact fp32 csq bitcast
            cnr = resid.tile([128, GW], bf16, tag="cnr")
            nc.vector.tensor_copy(cnr[:, 0:D], cn_t[:, :])
            nc.vector.tensor_copy(cnr[:, D:D + 2].bitcast(fp32),
                                  csq_col[:, :])
            nc.vector.memset(cnr[:, D + 2:GW], 0.0)
            nc.sync.dma_start(out=gbuf[:, :], in_=cnr[:, :])

            # local transposes of this core's CnT blocks -> AG payload
            stage = resid.tile([128, 1024], fp8, tag="stage")
            for dj in range(8):
                tpl = psall.tile([128, 128], fp32, tag="tpl", bufs=2)
                nc.tensor.transpose(
                    out=tpl[:, :], in_=cn_t[:, dj * 128:(dj + 1) * 128],
                    identity=ident_f[:, :])
                if dj % 2 == 0:
                    nc.scalar.copy(stage[:, dj * 128:(dj + 1) * 128],
                                   tpl[:, :])
                else:
                    nc.vector.tensor_copy(stage[:, dj * 128:(dj + 1) * 128],
                                          tpl[:, :])
            myT = resid.tile([128, 1024], fp8, tag="myT")
            nc.vector.tensor_scalar_mul(myT[:], stage[:, :], -2.0)
            csq_bf = small.tile([1, 128], fp8, tag="csq_bf")
            tpc = psall.tile([1, 128], fp32, tag="tpc", bufs=1)
            nc.tensor.transpose(out=tpc[:1, :], in_=csq_col[:, :1],
                                identity=ident_f[:, :])
            nc.vector.tensor_scalar(out=csq_bf[:1, :], in0=tpc[:1, :],
                                    scalar1=0.25, scalar2=None,
                                    op0=ALU.mult)
            nc.sync.dma_start(out=ag_in[0:128, 0:1024], in_=stage[:, :])
            nc.sync.dma_start(out=ag_in[128:129, 0:128], in_=csq_bf[:1, :])

            # ---- phase 4: the only collective - small fp8 AllGather ----
            nc.gpsimd.collective_compute(
                "AllGather", ALU.bypass,
                ins=[ag_in.opt()], outs=[cn_dram.opt()],
                replica_groups=[list(range(NCORES))])

            # ---- comm window: interleaved fa norms + intra per tile ----
            # f-branch: dots via fused stt (DVE) + exact bitcast csq
            # fa-branch: diff (DVE 2x) + Square-accumulate (ACT)
            dots_f = resid.tile([128, nbt], fp32, tag="dots_f")
            ssa_col = resid.tile([128, nbt], fp32, tag="ssa_col")
            csqg = resid.tile([128, nbt], fp32, tag="csqg")
            for b in range(nbt):
                r0 = b * 128
                xa_t = stream.tile([128, D], bf16, tag="xain", bufs=3)
                nc.sync.dma_start(out=xa_t[:], in_=feat_adv[r0:r0 + 128, :])
                ssa = small.tile([128, 1], fp32, tag="ss")
                scr = stream.tile([128, D], fp32, tag="scrA")
                nc.scalar.activation(out=scr[:], in_=xa_t[:],
                                     func=AF.Square, accum_out=ssa[:])
                nrma = small.tile([128, 1], fp32, tag="nrm")
                nc.scalar.activation(out=nrma[:], in_=ssa[:], func=AF.Sqrt)
                nc.vector.tensor_scalar_max(nrma[:], nrma[:], 1e-12)
                rina = small.tile([128, 1], fp32, tag="rin")
                nc.vector.reciprocal(rina[:], nrma[:])
                xa_bf = stream.tile([128, D], bf16, tag="xab", bufs=3)
                nc.vector.tensor_scalar_mul(xa_bf[:], xa_t[:], rina[:, :1])

                g_t = stream.tile([128, GW], bf16, tag="gat", bufs=6)
                nc.gpsimd.indirect_dma_start(
                    out=g_t[:], out_offset=None, in_=gbuf[:, :],
                    in_offset=bass.IndirectOffsetOnAxis(
                        ap=lg_tiles[b][:, :1], axis=0))
                prodf = stream.tile([128, D], bf16, tag="pdump", bufs=3)
                nc.vector.scalar_tensor_tensor(
                    out=prodf[:], in0=f_tiles[b][:, 0:D], scalar=1.0,
                    in1=g_t[:, 0:D], op0=ALU.mult, op1=ALU.mult,
                    accum_out=dots_f[:, b:b + 1])
                da_t = stream.tile([128, D], bf16, tag="pdump", bufs=3)
                nc.vector.tensor_sub(da_t[:], xa_bf[:], g_t[:, 0:D])
                sqd = stream.tile([128, D], bf16, tag="adump", bufs=3)
                nc.scalar.activation(out=sqd[:], in_=da_t[:],
                                     func=AF.Square,
                                     accum_out=ssa_col[:, b:b + 1])
                nc.vector.tensor_copy(csqg[:, b:b + 1],
                                      g_t[:, D:D + 2].bitcast(fp32))

            # intra finalize: d^2 = mask - 2 dot + csq_g; mask the dist
            base = small.tile([128, nbt], fp32, tag="base")
            nc.vector.tensor_add(base[:], csqg[:], mask_nb[:])
            ssf_t = small.tile([128, nbt], fp32, tag="ssf_t")
            nc.vector.scalar_tensor_tensor(
                out=ssf_t[:], in0=dots_f[:], scalar=-2.0, in1=base[:],
                op0=ALU.mult, op1=ALU.add)
            nc.vector.tensor_scalar_max(ssf_t[:], ssf_t[:], 0.0)
            dist_f = small.tile([128, nbt], fp32, tag="dist_f")
            nc.scalar.activation(out=dist_f[:], in_=ssf_t[:], func=AF.Sqrt)
            dist_a = small.tile([128, nbt], fp32, tag="dist_a")
            nc.scalar.activation(out=dist_a[:], in_=ssa_col[:], func=AF.Sqrt)
            nc.vector.tensor_mul(dist_f[:], dist_f[:], mask_nb[:])
            nc.vector.tensor_mul(dist_a[:], dist_a[:], mask_nb[:])
            ir_f = small.tile([128, 1], fp32, tag="ir_f")
            nc.vector.tensor_reduce(out=ir_f[:], in_=dist_f[:], axis=AX.X,
                                    op=ALU.add)
            ir_a = small.tile([128, 1], fp32, tag="ir_a")
            nc.vector.tensor_reduce(out=ir_a[:], in_=dist_a[:], axis=AX.X,
                                    op=ALU.add)
            intra_rows = small.tile([128, 1], fp32, tag="intra_rows")
            nc.vector.tensor_add(intra_rows[:], ir_f[:], ir_a[:])

            # ---- phase 6: pairwise inter from the AllGather ----
            # per-rank stage loads pipelined with per-rank matmul groups
            rk = cn_dram[:, :].rearrange("(k r) j -> r k j", k=NCORES)
            csq_row = constp.tile([1, 1024], fp8, tag="csq_row")
            nc.sync.dma_start(out=csq_row[:],
                              in_=rk[128:129, :, 0:128])
            stg_tiles = []
            for r in range(NCORES):
                stg_r = resid.tile([128, 1024], fp8, tag=f"stg{r}",
                                   name=f"stg{r}")
                eng = (nc.sync, nc.gpsimd, nc.scalar)[r % 3]
                eng.dma_start(out=stg_r[:, :], in_=rk[0:128, r:r + 1, 0:1024])
                stg_tiles.append(stg_r)

            g_ps = psall.tile([128, C], fp32, tag="gmm", bufs=1)
            for r in range(NCORES):
                n0 = r * 128
                nsz = min(128, C - n0)
                for dj in range(8):
                    nc.tensor.matmul(
                        g_ps[:, n0:n0 + nsz],
                        lhsT=myT[:, dj * 128:(dj + 1) * 128],
                        rhs=stg_tiles[r][:, dj * 128:dj * 128 + nsz],
                        start=(dj == 0), stop=False)
                nc.tensor.matmul(g_ps[:, n0:n0 + nsz],
                                 lhsT=ones_row[:1, :],
                                 rhs=csq_row[:1, n0:n0 + nsz],
                                 start=False, stop=True)

            # halves pipelined through DVE (d2b) -> ACT (sqrt, relu+acc)
            inter_h = small.tile([128, 2], fp32, tag="inter_h")
            for hi, (h0, hsz) in enumerate(((0, 512), (512, C - 512))):
                d2b = stream.tile([128, 512], fp32, tag="d2b", bufs=2)
                nc.vector.tensor_scalar(
                    out=d2b[:, 0:hsz], in0=g_ps[:, h0:h0 + hsz],
                    scalar1=csq_col[:, :1],
                    scalar2=0.0, op0=ALU.add, op1=ALU.max)
                dst = stream.tile([128, 512], fp32, tag="dsth", bufs=2)
                nc.scalar.activation(out=dst[:, 0:hsz], in_=d2b[:, 0:hsz],
                                     func=AF.Sqrt)
                term = stream.tile([128, 512], fp32, tag="termh", bufs=2)
                nc.scalar.activation(out=term[:, 0:hsz], in_=dst[:, 0:hsz],
                                     func=AF.Relu, bias=1.0, scale=-1.0,
                                     accum_out=inter_h[:, hi:hi + 1])
            inter_rows = small.tile([128, 1], fp32, tag="inter_rows")
            nc.vector.tensor_add(inter_rows[:], inter_h[:, 0:1],
                                 inter_h[:, 1:2])

            # ---- phase 7: final reduce + partials out ----
            partials = small.tile([128, 2], fp32, tag="partials")
            nc.vector.memset(partials[:], 0.0)
            nc.vector.tensor_copy(partials[:, 0:1], intra_rows[:])
            nc.vector.tensor_copy(partials[:, 1:2], inter_rows[:, :])
            pr = small.tile([1, 2], fp32, tag="pr")
            nc.gpsimd.tensor_reduce(out=pr[:1, :], in_=partials[:, :],
                                    axis=AX.C, op=ALU.add)
            nc.sync.dma_start(out=out[0:1, 0:2], in_=pr[:1, :])

    nc.compile()
    return nc


def _get_nc(nbt):
    key = ("nc", nbt)
    if key not in _state:
        _state[key] = _build(nbt)
    return _state[key]


def kernel(features, features_adv, centers, labels):
    from concourse import bass_utils
    import ml_dtypes

    labels_np = np.asarray(labels).astype(np.int64).reshape(-1)
    own = (labels_np >> 7).astype(np.int64)
    counts = np.bincount(own, minlength=NCORES)
    nbt = int(np.ceil(max(int(counts.max()), 1) / 128.0))
    bpc = nbt * 128
    nc = _get_nc(nbt)

    features_bf = np.asarray(features, dtype=np.float32).astype(
        ml_dtypes.bfloat16)
    features_adv_bf = np.asarray(features_adv, dtype=np.float32).astype(
        ml_dtypes.bfloat16)
    centers_np = np.asarray(centers, dtype=np.float32)
    centers_pad = np.zeros((CPAD, D), dtype=np.float32)
    centers_pad[:C] = centers_np

    in_maps = []
    for k in range(NCORES):
        idx = np.nonzero(own == k)[0]
        nk = len(idx)
        fk = np.zeros((bpc, D), dtype=ml_dtypes.bfloat16)
        fk[:nk] = features_bf[idx]
        fak = np.zeros((bpc, D), dtype=ml_dtypes.bfloat16)
        fak[:nk] = features_adv_bf[idx]
        loc = (labels_np[idx] - 128 * k).astype(np.int32)
        labk = np.full((bpc, 1), -1, dtype=np.int32)
        labk[:nk, 0] = loc
        lgk = np.zeros((bpc,), dtype=np.int32)
        lgk[:nk] = loc
        in_maps.append({
            "features": fk,
            "features_adv": fak,
            "centers_sh": np.ascontiguousarray(
                centers_pad[k * 128:(k + 1) * 128]),
            "labels": labk,
            "labels_g": np.ascontiguousarray(lgk.reshape(nbt, 128).T),
        })

    res = bass_utils.run_bass_kernel_spmd(
        nc, in_maps, core_ids=list(range(NCORES)),
        trace=bool(int(os.environ.get("AFD_TRACE", "0"))))
    _state["last_results"] = res
    parts = np.stack([res.results[k]["out"][0] for k in range(NCORES)])
    intra_sum = float(parts[:, 0].sum())
    inter_sum = float(parts[:, 1].sum())
    val = intra_sum / B - 0.25 * (inter_sum - C) / N_PAIRS
    return np.asarray(np.float32(val))


# revision 10
# speedup vs baseline: 2.2406x; 1.1139x over previous
"""AFD loss kernel for 8 TRN2 NeuronCores (Bass/Tile).

Algorithm (matches the reference loss_fn):
  f  = l2norm(features); fa = l2norm(features_adv)
  per-class sums/counts of f via one-hot matmul
  centers_new = where(counts>0, 0.9*centers + 0.1*sums/max(counts,1), centers)
  intra = mean ||f - centers_new[labels]|| + mean ||fa - centers_new[labels]||
        with ||x - c||^2 = ||x||^2 - 2 x.c + ||c||^2    (x unit-norm)
  inter = sum_{i<j} relu(1 - ||ci - cj||) / n_pairs   (symmetric full-sum trick)
  loss  = intra - 0.5 * inter

Structure (v14, label-sharded):
  - the batch is sharded BY LABEL OWNERSHIP on the host: core k receives
    exactly the samples whose label is in [128k, 128k+128), padded with
    zero rows (one-hot label -1 -> all-zero row; masked out of intra).
    Segment sums/counts are then fully LOCAL - no cross-core reduction
    collective at all.  The momentum update runs locally in exact fp32.
  - one-hot matmul shrinks to [128 batch x 128 local classes] per tile
    (~8x less PE work than all-class segment sums); the count column is
    fused as column 1024 of the f tiles (ones column)
  - intra: updated center rows (bf16) + exact fp32 csq (bitcast into two
    bf16 columns) are written to a LOCAL DRAM row buffer; per-tile
    indirect gathers + DVE products (2x mode) + ACT accumulation; per-row
    validity mask applied after the sqrt
  - the ONLY collective is a small fp8 AllGather (129 rows/rank: locally
    PE-transposed CnT blocks + 0.25x-scaled csq row, recovered by a
    4.0-valued ones-row matmul) feeding the pairwise inter block
  - inter: per-rank column blocks of -2*Cn_my @ Cn.T from one contiguous
    stage load; zero pad rows/cols provably contribute 0; symmetric
    full-sum with the C diagonal terms removed in the final formula
  - per-core [intra_sum, inter_sum] partials; host sums 8x2 floats and
    applies the affine formula (the unshard step)
"""

import os
from contextlib import ExitStack

import numpy as np

NCORES = 8
B = 8192
D = 1024
C = 1000
MOM = 0.9
N_PAIRS = C * (C - 1) / 2.0
CPAD = 1024                 # classes padded to full chunks
GW = D + 4                  # gather row: D bf16 + csq (f32 as 2 bf16) + pad
SW = D + 8                  # AG stage row width (fp8)
AGR = 129                   # AG rows/rank: 128 cnT-stage + 1 csq
RW = D + 1                  # reduce row width: sums + count column

_state = {}


def _build(nbt):
    import concourse.bacc as bacc
    import concourse.bass as bass
    import concourse.mybir as mybir
    import concourse.tile as tile
    from concourse.masks import make_identity

    fp32 = mybir.dt.float32
    bf16 = mybir.dt.bfloat16
    fp8 = mybir.dt.float8e4
    i32 = mybir.dt.int32
    AF = mybir.ActivationFunctionType
    ALU = mybir.AluOpType
    AX = mybir.AxisListType

    bpc = nbt * 128

    nc = bacc.Bacc("TRN2", target_bir_lowering=False, debug=False,
                   num_devices=NCORES)

    feat = nc.dram_tensor("features", [bpc, D], bf16, kind="ExternalInput")
    feat_adv = nc.dram_tensor("features_adv", [bpc, D], bf16,
                              kind="ExternalInput")
    centers_sh = nc.dram_tensor("centers_sh", [128, D], fp32,
                                kind="ExternalInput")
    labels = nc.dram_tensor("labels", [bpc, 1], i32, kind="ExternalInput")
    labels_g = nc.dram_tensor("labels_g", [128, nbt], i32,
                              kind="ExternalInput")
    out = nc.dram_tensor("out", [1, 2], fp32, kind="ExternalOutput")

    with tile.TileContext(nc) as tc:
        with (
            tc.tile_pool(name="const", bufs=1) as constp,
            tc.tile_pool(name="resid", bufs=1) as resid,
            tc.tile_pool(name="stream", bufs=2) as stream,
            tc.tile_pool(name="small", bufs=8) as small,
            tc.tile_pool(name="psall", bufs=1, space="PSUM") as psall,
            tc.tile_pool(name="dram", bufs=1, space="DRAM") as dram,
        ):
            # ---- constants ----
            iota_t = constp.tile([128, 128], fp32, tag="iota")
            nc.gpsimd.iota(iota_t[:], pattern=[[1, 128]], base=0,
                           channel_multiplier=0,
                           allow_small_or_imprecise_dtypes=True)
            ones_row = constp.tile([1, 128], fp8, tag="ones_row")
            nc.vector.memset(ones_row[:], 4.0)
            ident_f = constp.tile([128, 128], fp32, tag="ident_f")
            make_identity(nc, ident_f[:])

            # DRAM bounces
            gbuf = dram.tile([128, GW], bf16, tag="gbuf")
            ag_in = dram.tile([AGR, SW], fp8, tag="ag_in")
            cn_dram = dram.tile([AGR * NCORES, SW], fp8, tag="cn",
                                addr_space="Shared")

            # ---- phase 1: load + normalize f; local one-hot; mask ----
            f_tiles, lab_tiles, x_tiles = [], [], []
            mask_nb = resid.tile([128, nbt], fp32, tag="mask_nb")
            for b in range(nbt):
                r0 = b * 128
                x_t = stream.tile([128, D], bf16, tag="xin", bufs=3,
                                  name=f"x{b}")
                nc.sync.dma_start(out=x_t[:], in_=feat[r0:r0 + 128, :])
                x_tiles.append(x_t)
                lab_t = resid.tile([128, 1], i32, tag=f"lab{b}",
                                   name=f"lab{b}")
                nc.sync.dma_start(out=lab_t[:], in_=labels[r0:r0 + 128, :])
                lab_tiles.append(lab_t)
            cen = resid.tile([128, D], fp32, tag="cen")
            nc.sync.dma_start(out=cen[:, :], in_=centers_sh[:, :])
            lg_tiles = []
            for b in range(nbt):
                lg_t = resid.tile([128, 1], i32, tag=f"lg{b}",
                                  name=f"lg{b}")
                nc.sync.dma_start(out=lg_t[:], in_=labels_g[:, b:b + 1])
                lg_tiles.append(lg_t)

            oh_tiles = []
            for b in range(nbt):
                x_t = x_tiles[b]
                ss = small.tile([128, 1], fp32, tag="ss")
                scr = stream.tile([128, D], fp32, tag="scrB")
                nc.scalar.activation(out=scr[:], in_=x_t[:],
                                     func=AF.Square, accum_out=ss[:])
                nrm = small.tile([128, 1], fp32, tag="nrm")
                nc.scalar.activation(out=nrm[:], in_=ss[:], func=AF.Sqrt)
                nc.vector.tensor_scalar_max(nrm[:], nrm[:], 1e-12)
                rin = small.tile([128, 1], fp32, tag="rin")
                nc.vector.reciprocal(rin[:], nrm[:])
                f_t = resid.tile([128, RW], bf16, tag=f"f{b}",
                                 name=f"f{b}")
                nc.vector.tensor_scalar_mul(f_t[:, 0:D], x_t[:],
                                            rin[:, :1])
                nc.vector.memset(f_t[:, D:RW], 1.0)
                f_tiles.append(f_t)
                lab_f = small.tile([128, 1], fp32, tag="labf")
                nc.scalar.copy(lab_f[:], lab_tiles[b][:])
                nc.vector.tensor_scalar(
                    out=mask_nb[:, b:b + 1], in0=lab_f[:], scalar1=-0.5,
                    scalar2=None, op0=ALU.is_gt)
                oh_t = resid.tile([128, 128], bf16, tag=f"oh{b}",
                                  name=f"oh{b}")
                nc.vector.tensor_scalar(
                    out=oh_t[:], in0=iota_t[:], scalar1=lab_f[:, :1],
                    scalar2=None, op0=ALU.is_equal)
                oh_tiles.append(oh_t)

            # ---- phase 2: local segment sums + fused counts column ----
            ps = psall.tile([128, RW], fp32, tag="segsum", bufs=1)
            for b in range(nbt):
                st, sp = (b == 0), (b == nbt - 1)
                for n0, nsz in ((0, 512), (512, 512), (1024, 1)):
                    nc.tensor.matmul(
                        ps[:, n0:n0 + nsz],
                        lhsT=oh_tiles[b][:, :],
                        rhs=f_tiles[b][:, n0:n0 + nsz],
                        start=st, stop=sp)

            # ---- phase 3: momentum update (local, exact fp32) ----
            csq_col = small.tile([128, 1], fp32, tag="csq_col")
            cntc = small.tile([128, 1], fp32, tag="cntc")
            nc.vector.tensor_scalar_max(cntc[:], ps[:, D:D + 1], 1.0)
            rcv = small.tile([128, 1], fp32, tag="rcv")
            nc.vector.reciprocal(rcv[:], cntc[:])
            w = small.tile([128, 1], fp32, tag="w")
            nc.vector.tensor_scalar(out=w[:], in0=ps[:, D:D + 1],
                                    scalar1=0.0, scalar2=1.0 - MOM,
                                    op0=ALU.is_gt, op1=ALU.mult)
            m = small.tile([128, 1], fp32, tag="m")
            nc.vector.tensor_tensor(out=m[:], in0=w[:], in1=rcv[:],
                                    op=ALU.mult)
            u = small.tile([128, 1], fp32, tag="u")
            nc.vector.tensor_scalar(out=u[:], in0=w[:], scalar1=-1.0,
                                    scalar2=1.0, op0=ALU.mult, op1=ALU.add)
            t1 = stream.tile([128, D], fp32, tag="scrC")
            nc.scalar.mul(t1[:], cen[:, :], u[:, :1])
            cn_t = resid.tile([128, D], fp32, tag="cn_t")
            nc.vector.scalar_tensor_tensor(
                out=cn_t[:, :], in0=ps[:, 0:D], scalar=m[:, :1],
                in1=t1[:, :], op0=ALU.mult, op1=ALU.add)
            scr2 = stream.tile([128, D], bf16, tag="sqdump")
            nc.scalar.activation(out=scr2[:], in_=cn_t[:, :],
                                 func=AF.Square, accum_out=csq_col[:])
            # local gather-row buffer: bf16 rows + exact fp32 csq bitcast
            cnr = resid.tile([128, GW], bf16, tag="cnr")
            nc.vector.tensor_copy(cnr[:, 0:D], cn_t[:, :])
            nc.vector.tensor_copy(cnr[:, D:D + 2].bitcast(fp32),
                                  csq_col[:, :])
            nc.vector.memset(cnr[:, D + 2:GW], 0.0)
            nc.sync.dma_start(out=gbuf[:, :], in_=cnr[:, :])

            # local transposes of this core's CnT blocks -> AG payload
            stage = resid.tile([128, 1024], fp8, tag="stage")
            for dj in range(8):
                tpl = psall.tile([128, 128], fp32, tag="tpl", bufs=2)
                nc.tensor.transpose(
                    out=tpl[:, :], in_=cn_t[:, dj * 128:(dj + 1) * 128],
                    identity=ident_f[:, :])
                if dj % 2 == 0:
                    nc.scalar.copy(stage[:, dj * 128:(dj + 1) * 128],
                                   tpl[:, :])
                else:
                    nc.vector.tensor_copy(stage[:, dj * 128:(dj + 1) * 128],
                                          tpl[:, :])
            myT = resid.tile([128, 1024], fp8, tag="myT")
            nc.vector.tensor_scalar_mul(myT[:], stage[:, :], -2.0)
            csq_bf = small.tile([1, 128], fp8, tag="csq_bf")
            tpc = psall.tile([1, 128], fp32, tag="tpc", bufs=1)
            nc.tensor.transpose(out=tpc[:1, :], in_=csq_col[:, :1],
                                identity=ident_f[:, :])
            nc.vector.tensor_scalar(out=csq_bf[:1, :], in0=tpc[:1, :],
                                    scalar1=0.25, scalar2=None,
                                    op0=ALU.mult)
            nc.sync.dma_start(out=ag_in[0:128, 0:1024], in_=stage[:, :])
            nc.sync.dma_start(out=ag_in[128:129, 0:128], in_=csq_bf[:1, :])

            # ---- phase 4: the only collective - small fp8 AllGather ----
            nc.gpsimd.collective_compute(
                "AllGather", ALU.bypass,
                ins=[ag_in.opt()], outs=[cn_dram.opt()],
                replica_groups=[list(range(NCORES))])

            # ---- comm window: interleaved fa norms + intra per tile ----
            # f-branch: dots via fused stt (DVE) + exact bitcast csq
            # fa-branch: diff (DVE 2x) + Square-accumulate (ACT)
            dots_f = resid.tile([128, nbt], fp32, tag="dots_f")
            ssa_col = resid.tile([128, nbt], fp32, tag="ssa_col")
            csqg = resid.tile([128, nbt], fp32, tag="csqg")
            for b in range(nbt):
                r0 = b * 128
                xa_t = stream.tile([128, D], bf16, tag="xain", bufs=3)
                nc.sync.dma_start(out=xa_t[:], in_=feat_adv[r0:r0 + 128, :])
                ssa = small.tile([128, 1], fp32, tag="ss")
                scr = stream.tile([128, D], fp32, tag="scrA")
                nc.scalar.activation(out=scr[:], in_=xa_t[:],
                                     func=AF.Square, accum_out=ssa[:])
                nrma = small.tile([128, 1], fp32, tag="nrm")
                nc.scalar.activation(out=nrma[:], in_=ssa[:], func=AF.Sqrt)
                nc.vector.tensor_scalar_max(nrma[:], nrma[:], 1e-12)
                rina = small.tile([128, 1], fp32, tag="rin")
                nc.vector.reciprocal(rina[:], nrma[:])
                xa_bf = stream.tile([128, D], bf16, tag="xab", bufs=3)
                nc.vector.tensor_scalar_mul(xa_bf[:], xa_t[:], rina[:, :1])

                g_t = stream.tile([128, GW], bf16, tag="gat", bufs=6)
                nc.gpsimd.indirect_dma_start(
                    out=g_t[:], out_offset=None, in_=gbuf[:, :],
                    in_offset=bass.IndirectOffsetOnAxis(
                        ap=lg_tiles[b][:, :1], axis=0))
                prodf = stream.tile([128, D], bf16, tag="pdump", bufs=6)
                nc.vector.scalar_tensor_tensor(
                    out=prodf[:], in0=f_tiles[b][:, 0:D], scalar=1.0,
                    in1=g_t[:, 0:D], op0=ALU.mult, op1=ALU.mult,
                    accum_out=dots_f[:, b:b + 1])
                da_t = stream.tile([128, D], bf16, tag="pdump", bufs=6)
                nc.vector.tensor_sub(da_t[:], xa_bf[:], g_t[:, 0:D])
                sqd = stream.tile([128, D], bf16, tag="adump", bufs=6)
                nc.scalar.activation(out=sqd[:], in_=da_t[:],
                                     func=AF.Square,
                                     accum_out=ssa_col[:, b:b + 1])
                nc.vector.tensor_copy(csqg[:, b:b + 1],
                                      g_t[:, D:D + 2].bitcast(fp32))

            # intra finalize: d^2 = mask - 2 dot + csq_g; mask the dist
            base = small.tile([128, nbt], fp32, tag="base")
            nc.vector.tensor_add(base[:], csqg[:], mask_nb[:])
            ssf_t = small.tile([128, nbt], fp32, tag="ssf_t")
            nc.vector.scalar_tensor_tensor(
                out=ssf_t[:], in0=dots_f[:], scalar=-2.0, in1=base[:],
                op0=ALU.mult, op1=ALU.add)
            nc.vector.tensor_scalar_max(ssf_t[:], ssf_t[:], 0.0)
            dist_f = small.tile([128, nbt], fp32, tag="dist_f")
            nc.scalar.activation(out=dist_f[:], in_=ssf_t[:], func=AF.Sqrt)
            dist_a = small.tile([128, nbt], fp32, tag="dist_a")
            nc.scalar.activation(out=dist_a[:], in_=ssa_col[:], func=AF.Sqrt)
            nc.vector.tensor_mul(dist_f[:], dist_f[:], mask_nb[:])
            nc.vector.tensor_mul(dist_a[:], dist_a[:], mask_nb[:])
            ir_f = small.tile([128, 1], fp32, tag="ir_f")
            nc.vector.tensor_reduce(out=ir_f[:], in_=dist_f[:], axis=AX.X,
                                    op=ALU.add)
            ir_a = small.tile([128, 1], fp32, tag="ir_a")
            nc.vector.tensor_reduce(out=ir_a[:], in_=dist_a[:], axis=AX.X,
                                    op=ALU.add)
            intra_rows = small.tile([128, 1], fp32, tag="intra_rows")
            nc.vector.tensor_add(intra_rows[:], ir_f[:], ir_a[:])

            # ---- phase 6: pairwise inter from the AllGather ----
            # per-rank stage loads pipelined with per-rank matmul groups
            rk = cn_dram[:, :].rearrange("(k r) j -> r k j", k=NCORES)
            csq_row = constp.tile([1, 1024], fp8, tag="csq_row")
            nc.sync.dma_start(out=csq_row[:],
                              in_=rk[128:129, :, 0:128])
            stg_tiles = []
            for r in range(NCORES):
                stg_r = resid.tile([128, 1024], fp8, tag=f"stg{r}",
                                   name=f"stg{r}")
                eng = (nc.sync, nc.gpsimd, nc.scalar)[r % 3]
                eng.dma_start(out=stg_r[:, :], in_=rk[0:128, r:r + 1, 0:1024])
                stg_tiles.append(stg_r)

            # dj-outer so each myT chunk stays loaded as the stationary
            # weight for 8 consecutive matmuls (8 weight loads, not 64)
            g_ps = psall.tile([128, C], fp32, tag="gmm", bufs=1)
            for dj in range(8):
                for r in range(NCORES):
                    n0 = r * 128
                    nsz = min(128, C - n0)
                    nc.tensor.matmul(
                        g_ps[:, n0:n0 + nsz],
                        lhsT=myT[:, dj * 128:(dj + 1) * 128],
                        rhs=stg_tiles[r][:, dj * 128:dj * 128 + nsz],
                        start=(dj == 0), stop=False)
            for r in range(NCORES):
                n0 = r * 128
                nsz = min(128, C - n0)
                nc.tensor.matmul(g_ps[:, n0:n0 + nsz],
                                 lhsT=ones_row[:1, :],
                                 rhs=csq_row[:1, n0:n0 + nsz],
                                 start=False, stop=True)

            # halves pipelined through DVE (d2b) -> ACT (sqrt, relu+acc)
            inter_h = small.tile([128, 2], fp32, tag="inter_h")
            for hi, (h0, hsz) in enumerate(((0, 512), (512, C - 512))):
                d2b = stream.tile([128, 512], fp32, tag="d2b", bufs=2)
                nc.vector.tensor_scalar(
                    out=d2b[:, 0:hsz], in0=g_ps[:, h0:h0 + hsz],
                    scalar1=csq_col[:, :1],
                    scalar2=0.0, op0=ALU.add, op1=ALU.max)
                dst = stream.tile([128, 512], fp32, tag="dsth", bufs=2)
                nc.scalar.activation(out=dst[:, 0:hsz], in_=d2b[:, 0:hsz],
                                     func=AF.Sqrt)
                term = stream.tile([128, 512], fp32, tag="termh", bufs=2)
                nc.scalar.activation(out=term[:, 0:hsz], in_=dst[:, 0:hsz],
                                     func=AF.Relu, bias=1.0, scale=-1.0,
                                     accum_out=inter_h[:, hi:hi + 1])
            inter_rows = small.tile([128, 1], fp32, tag="inter_rows")
            nc.vector.tensor_add(inter_rows[:], inter_h[:, 0:1],
                                 inter_h[:, 1:2])

            # ---- phase 7: final reduce + partials out ----
            partials = small.tile([128, 2], fp32, tag="partials")
            nc.vector.memset(partials[:], 0.0)
            nc.vector.tensor_copy(partials[:, 0:1], intra_rows[:])
            nc.vector.tensor_copy(partials[:, 1:2], inter_rows[:, :])
            pr = small.tile([1, 2], fp32, tag="pr")
            nc.gpsimd.tensor_reduce(out=pr[:1, :], in_=partials[:, :],
                                    axis=AX.C, op=ALU.add)
            nc.sync.dma_start(out=out[0:1, 0:2], in_=pr[:1, :])

    nc.compile()
    return nc


def _get_nc(nbt):
    key = ("nc", nbt)
    if key not in _state:
        _state[key] = _build(nbt)
    return _state[key]


def kernel(features, features_adv, centers, labels):
    from concourse import bass_utils
    import ml_dtypes

    labels_np = np.asarray(labels).astype(np.int64).reshape(-1)
    own = (labels_np >> 7).astype(np.int64)
    counts = np.bincount(own, minlength=NCORES)
    nbt = int(np.ceil(max(int(counts.max()), 1) / 128.0))
    bpc = nbt * 128
    nc = _get_nc(nbt)

    features_bf = np.asarray(features, dtype=np.float32).astype(
        ml_dtypes.bfloat16)
    features_adv_bf = np.asarray(features_adv, dtype=np.float32).astype(
        ml_dtypes.bfloat16)
    centers_np = np.asarray(centers, dtype=np.float32)
    centers_pad = np.zeros((CPAD, D), dtype=np.float32)
    centers_pad[:C] = centers_np

    in_maps = []
    for k in range(NCORES):
        idx = np.nonzero(own == k)[0]
        nk = len(idx)
        fk = np.zeros((bpc, D), dtype=ml_dtypes.bfloat16)
        fk[:nk] = features_bf[idx]
        fak = np.zeros((bpc, D), dtype=ml_dtypes.bfloat16)
        fak[:nk] = features_adv_bf[idx]
        loc = (labels_np[idx] - 128 * k).astype(np.int32)
        labk = np.full((bpc, 1), -1, dtype=np.int32)
        labk[:nk, 0] = loc
        lgk = np.zeros((bpc,), dtype=np.int32)
        lgk[:nk] = loc
        in_maps.append({
            "features": fk,
            "features_adv": fak,
            "centers_sh": np.ascontiguousarray(
                centers_pad[k * 128:(k + 1) * 128]),
            "labels": labk,
            "labels_g": np.ascontiguousarray(lgk.reshape(nbt, 128).T),
        })

    res = bass_utils.run_bass_kernel_spmd(
        nc, in_maps, core_ids=list(range(NCORES)),
        trace=bool(int(os.environ.get("AFD_TRACE", "0"))))
    _state["last_results"] = res
    parts = np.stack([res.results[k]["out"][0] for k in range(NCORES)])
    intra_sum = float(parts[:, 0].sum())
    inter_sum = float(parts[:, 1].sum())
    val = intra_sum / B - 0.25 * (inter_sum - C) / N_PAIRS
    return np.asarray(np.float32(val))
